# revision 44
# baseline (speedup 1.0000x reference)
"""Multi-head attention TRN2 Bass kernel (8 NeuronCores, SPMD).

Problem: B=2, S=2048, EMB=1024, H=16, DK=DV=64.
Sharding: core c -> (batch b = c//4, head-group g = c%4 of 4 heads).
Host pre-shards + pre-transposes inputs (x^T, keep-mask^T, W slices in f16);
each core computes its 4 heads fully on-chip:

  Q^T/K^T = W^T x^T (f16 matmuls, f32 psum, + per-partition bias)
  V       = x_kv W_v + bv   (stored per-128-row chunk, with a ones column)
  S^T[k,q] chunk = K^T_chunk^T @ Q^T  -> psum f32
  E^T = exp(S/8 - 4) (ScalarE, psum->sbuf f16), then E^T *= keep^T (VectorE)
  z^T[dv+1, q] = sum_kc [V_kc | 1]^T @ E^T_kc  (N=512 matmuls; ones row
      gives the softmax denominator for free)
  z = transpose(z^T) rows scaled by 1/denominator  (PE transpose + VectorE;
      softmax shift-invariance: the -4 bias cancels)

No max-subtraction (scores are O(1) sigma, the exp bias guards f16 range and
cancels in normalization). Emission order pipelines DMA -> projections ->
scores(qg0) and then PV(qg-1)/scores(qg) per-head so ScalarE (exp, the
1 el/lane/cycle floor) and the PE (matmul streaming) overlap throughout.
"""
import numpy as np
from contextlib import ExitStack

import concourse.bass as bass
import concourse.tile as tile
from concourse import bacc, mybir
from concourse.bass_utils import run_bass_kernel_spmd
from concourse import masks

f32 = mybir.dt.float32
f16 = mybir.dt.float16
AF = mybir.ActivationFunctionType

B, S, EMB, H, DK, DV = 2, 2048, 1024, 16, 64, 64
NCORE = 8
HPC = 4                 # heads per core
COLS = HPC * DK         # 256 projection cols per core
ECH = EMB // 128        # 8 emb chunks
KC = S // 128           # 16 key chunks
NQG = 4                 # q groups of 512
QGS = S // NQG          # 512
VROW = HPC * (DV + 1)   # 260: per-chunk V row layout [4 heads x (64 v | 1)]

TRACE = False           # set by test harness for profiling runs
_CACHE = {}


def _build():
    nc = bacc.Bacc()
    XQT = nc.declare_dram_parameter("xqt", [EMB, S], f16, isOutput=False)
    XKVT = nc.declare_dram_parameter("xkvt", [EMB, S], f16, isOutput=False)
    KEEPT = nc.declare_dram_parameter("keept", [S, S], f16, isOutput=False)
    WQ = nc.declare_dram_parameter("wq", [EMB, COLS], f16, isOutput=False)
    WK = nc.declare_dram_parameter("wk", [EMB, COLS], f16, isOutput=False)
    WV = nc.declare_dram_parameter("wv", [EMB, COLS], f16, isOutput=False)
    BQ = nc.declare_dram_parameter("bq", [COLS], f32, isOutput=False)
    BK = nc.declare_dram_parameter("bk", [COLS], f32, isOutput=False)
    BV = nc.declare_dram_parameter("bv", [COLS], f32, isOutput=False)
    OUT = nc.declare_dram_parameter("out", [S, COLS], f32, isOutput=True)

    with tile.TileContext(nc) as tc, ExitStack() as top:
        # ---- long-lived buffers -------------------------------------------
        persist = top.enter_context(tc.tile_pool(name="persist", bufs=1))
        # Q^T/K^T: per column-chunk cc (2 heads stacked: rows [hl*64, hl*64+64))
        qT = [persist.tile([128, S], f16, tag=f"qT{cc}", name=f"qT{cc}")
              for cc in range(2)]
        kT = [persist.tile([128, S], f16, tag=f"kT{cc}", name=f"kT{cc}")
              for cc in range(2)]
        # V (+ ones cols): [128 k-in-chunk, KC * (4 heads x 65)]
        v_sb = persist.tile([128, KC * VROW], f16, tag="v")
        nc.gpsimd.memset(v_sb[:], 1.0)  # ones columns survive at h*65+64
        bq_sb = persist.tile([128, 2], f32, tag="bq")
        bk_sb = persist.tile([128, 2], f32, tag="bk")
        bv_sb = persist.tile([1, COLS], f32, tag="bv")
        ebias = persist.tile([128, 1], f32, tag="ebias")
        nc.gpsimd.memset(ebias[:], -4.0)
        ident = persist.tile([128, 128], f16, tag="ident")
        masks.make_identity(nc, ident[:])
        # replicate bv across partitions: ones[1,128] (x) bv[1,256] via PE
        bv_h = persist.tile([1, COLS], f16, tag="bv_h")
        ones1 = persist.tile([1, 128], f16, tag="ones1")
        nc.gpsimd.memset(ones1[:], 1.0)
        bv_rep = persist.tile([128, COLS], f32, tag="bv_rep")

        # ---- attention-phase pools (opened early: scores for q-group 0 are
        # emitted inside the projection phase so ScalarE starts exp ASAP) ----
        kpool = top.enter_context(tc.tile_pool(name="kpool", bufs=2))
        epool = top.enter_context(tc.tile_pool(name="epool", bufs=2))
        zpool = top.enter_context(tc.tile_pool(name="zpool", bufs=8))
        ztpool = top.enter_context(tc.tile_pool(name="ztpool", bufs=3))
        rpool = top.enter_context(tc.tile_pool(name="rpool", bufs=4))
        aps_s = top.enter_context(
            tc.tile_pool(name="aps_s", bufs=2, space="PSUM")
        )

        keept_dram = KEEPT[:, :].rearrange("(kc p) q -> p kc q", p=128)
        v_v = v_sb[:].rearrange("p (kc r) -> p kc r", kc=KC)

        def load_keept(qg):
            qsl = slice(qg * QGS, (qg + 1) * QGS)
            keepT = kpool.tile([128, KC * QGS], f16, tag="keepT",
                               name=f"keepT{qg}")
            nc.sync.dma_start(
                keepT[:].rearrange("p (kc q) -> p kc q", kc=KC),
                keept_dram[:, :, qsl],
            )
            return keepT

        def scores_partial(qg, cc, eTp, keepT, kc_range):
            """QK^T -> exp -> mask for some k-chunks of one (q-group, pair).

            The two heads of column-chunk cc live at base partitions 0 and
            64 (distinct PE row groups): emitting their K=64 matmuls
            back-to-back into different psum banks makes them execute
            CONCURRENTLY on the array (HW row-tiling, probe3-measured ~2x).
            eTp is laid out [128, kc, hl, 512]."""
            qsl = slice(qg * QGS, (qg + 1) * QGS)
            for kc in kc_range:
                sp = aps_s.tile([128, 1024], f32, tag="sps",
                                name=f"sps{qg}{cc}{kc}")
                for hl in range(2):
                    rsl = slice(hl * 64, (hl + 1) * 64)
                    nc.tensor.matmul(
                        sp[:, hl * 512:(hl + 1) * 512],
                        kT[cc][rsl, kc * 128:(kc + 1) * 128],
                        qT[cc][rsl, qsl],
                        start=True, stop=True,
                    )
                esl = slice(kc * 1024, (kc + 1) * 1024)
                nc.scalar.activation(
                    eTp[:, esl], sp[:], AF.Exp, scale=0.125,
                    bias=ebias[:],
                )
                nc.vector.tensor_mul(
                    eTp[:, esl].rearrange("p (hl q) -> p hl q", hl=2),
                    eTp[:, esl].rearrange("p (hl q) -> p hl q", hl=2),
                    keepT[:, kc * QGS:(kc + 1) * QGS]
                    .rearrange("p (o q) -> p o q", o=1)
                    .broadcast_to([128, 2, QGS]),
                )

        def alloc_etp(qg, cc):
            return epool.tile([128, KC * 2 * QGS], f16, tag="eT",
                              name=f"eTp{qg}{cc}")

        def scores_pair(qg, cc, keepT):
            eTp = alloc_etp(qg, cc)
            scores_partial(qg, cc, eTp, keepT, range(KC))
            return eTp

        # ---- phase 1: projections (+ scores for q-group 0) ----------------
        with ExitStack() as pctx:
            wpool = pctx.enter_context(tc.tile_pool(name="wpool", bufs=1))
            xpool = pctx.enter_context(tc.tile_pool(name="xpool", bufs=1))
            xqpool = pctx.enter_context(tc.tile_pool(name="xqpool", bufs=2))
            pps = pctx.enter_context(
                tc.tile_pool(name="pps", bufs=2, space="PSUM")
            )
            ppsv = pctx.enter_context(
                tc.tile_pool(name="ppsv", bufs=2, space="PSUM")
            )

            w_sb = {}

            def load_w(name, W, ways=1):
                w = wpool.tile([128, ECH * COLS], f16, tag=name, name=name)
                wv_ = w[:].rearrange("p (ec c) -> p ec c", ec=ECH)
                cw = COLS // ways
                for i in range(ways):
                    nc.sync.dma_start(
                        wv_[:, :, i * cw:(i + 1) * cw],
                        W[:, i * cw:(i + 1) * cw]
                        .rearrange("(ec p) c -> p ec c", p=128),
                    )
                w_sb[name] = wv_

            xkvT = xpool.tile([128, ECH * S], f16, tag="xkvT")
            xkvT_v = xkvT[:].rearrange("p (ec s) -> p ec s", ec=ECH)

            # one DMA per 512-row s-chunk (all 8 emb chunks) so the first
            # K-projection unblocks after ~1 MiB instead of the full 4 MiB;
            # the first chunk is split further so the very first matmul in
            # the kernel starts after ~256 KiB
            def load_kv(sc, ways=1):
                ssl = slice(sc * 512, (sc + 1) * 512)
                ecw = ECH // ways
                for w in range(ways):
                    nc.sync.dma_start(
                        xkvT_v[:, w * ecw:(w + 1) * ecw, ssl],
                        XKVT[w * ecw * 128:(w + 1) * ecw * 128, ssl]
                        .rearrange("(ec p) s -> p ec s", p=128),
                    )

            def k_proj(sc):
                sl = slice(sc * 512, (sc + 1) * 512)
                for cc in range(2):
                    csl = slice(cc * 128, (cc + 1) * 128)
                    psk = pps.tile([128, 512], f32, tag="psqk",
                                   name=f"psk{sc}{cc}")
                    for ec in range(ECH):
                        nc.tensor.matmul(
                            psk[:], w_sb["wk"][:, ec, csl], xkvT_v[:, ec, sl],
                            start=(ec == 0), stop=(ec == ECH - 1),
                        )
                    nc.vector.tensor_scalar_add(
                        kT[cc][:, sl], psk[:], bk_sb[:, cc:cc + 1]
                    )

            def q_proj(sc):
                sl = slice(sc * 512, (sc + 1) * 512)
                xq = xqpool.tile([128, ECH * 512], f16, tag="xq",
                                 name=f"xq{sc}")
                xq_v = xq[:].rearrange("p (ec s) -> p ec s", ec=ECH)
                nc.sync.dma_start(
                    xq_v[:, :, :],
                    XQT[:, sl].rearrange("(ec p) s -> p ec s", p=128),
                )
                for cc in range(2):
                    csl = slice(cc * 128, (cc + 1) * 128)
                    psq = pps.tile([128, 512], f32, tag="psqk",
                                   name=f"psq{sc}{cc}")
                    for ec in range(ECH):
                        nc.tensor.matmul(
                            psq[:], w_sb["wq"][:, ec, csl], xq_v[:, ec, :],
                            start=(ec == 0), stop=(ec == ECH - 1),
                        )
                    nc.vector.tensor_scalar_add(
                        qT[cc][:, sl], psq[:], bq_sb[:, cc:cc + 1]
                    )

            def v_proj(sc, bv_b):
                for st in range(4):  # 128-row chunks within s-chunk
                    idx = sc * 4 + st
                    ssl = slice(idx * 128, (idx + 1) * 128)
                    psv = ppsv.tile([128, COLS], f32, tag="psv",
                                    name=f"psv{idx}")
                    for ec in range(ECH):
                        nc.tensor.matmul(
                            psv[:], xkvT_v[:, ec, ssl], w_sb["wv"][:, ec, :],
                            start=(ec == 0), stop=(ec == ECH - 1),
                        )
                    vdst = (
                        v_sb[:, idx * VROW:(idx + 1) * VROW]
                        .rearrange("p (h c) -> p h c", h=HPC)[:, :, 0:DV]
                    )
                    nc.vector.tensor_add(
                        vdst,
                        psv[:].rearrange("p (h c) -> p h c", h=HPC),
                        bv_b,
                    )

            load_w("wk", WK)
            nc.sync.dma_start(
                bk_sb[:], BK[:].rearrange("(cc p) -> p cc", p=128)
            )
            load_kv(0, ways=4)
            load_w("wq", WQ)
            nc.sync.dma_start(
                bq_sb[:], BQ[:].rearrange("(cc p) -> p cc", p=128)
            )
            k_proj(0)
            q_proj(0)
            # scores for q-group 0 trail each K-projection s-chunk by 4
            # k-chunks, so ScalarE starts exp ~15us earlier than waiting
            # for the full K^T
            keepT0 = load_keept(0)
            eTp0 = [alloc_etp(0, cc) for cc in range(2)]
            for cc in range(2):
                scores_partial(0, cc, eTp0[cc], keepT0, range(0, 4))
            for sc in range(1, 4):
                load_kv(sc)
                k_proj(sc)
                for cc in range(2):
                    scores_partial(0, cc, eTp0[cc], keepT0,
                                   range(4 * sc, 4 * sc + 4))
            eT_store = {0: eTp0}
            load_w("wv", WV)
            nc.sync.dma_start(
                bv_sb[:], BV[:].rearrange("(o c) -> o c", o=1)
            )

            for sc in range(1, 4):
                q_proj(sc)
            bv_ps = ppsv.tile([128, COLS], f32, tag="psv", name="bv_ps")
            nc.vector.tensor_copy(bv_h[:], bv_sb[:])
            nc.tensor.matmul(bv_ps[:], ones1[:], bv_h[:], start=True, stop=True)
            nc.scalar.activation(bv_rep[:], bv_ps[:], AF.Copy)
            bv_b = bv_rep[:].rearrange("p (h c) -> p h c", h=HPC)
            for sc in range(4):
                v_proj(sc, bv_b)

        # ---- phase 2: pipelined PV(qg-1) / scores(qg) ---------------------
        with ExitStack() as actx:
            aps_z = actx.enter_context(
                tc.tile_pool(name="aps_z", bufs=2, space="PSUM")
            )
            aps_n = actx.enter_context(
                tc.tile_pool(name="aps_n", bufs=2, space="PSUM")
            )

            def pv_h(qg, h, eTp, z_alls):
                # z^T[dv+1, q] = sum_kc V'_kc^T @ E^T_kc: 16 N=512 matmuls
                # per (qg, h) instead of 64 N=65 ones (PE dispatch-bound
                # otherwise), then a small PE transpose back + normalize.
                # eTp is the pair tile [128, kc, hl, 512] for cc = h//2.
                hl = h % 2
                zt = aps_z.tile([128, QGS], f32, tag="zt", name=f"zt{qg}{h}")
                for kc in range(KC):
                    off = (kc * 2 + hl) * QGS
                    nc.tensor.matmul(
                        zt[0:65, :],
                        v_v[:, kc, h * 65:(h + 1) * 65],
                        eTp[:, off:off + QGS],
                        start=(kc == 0), stop=(kc == KC - 1),
                    )
                zt_sb = ztpool.tile([128, QGS], f16, tag="ztsb",
                                    name=f"ztsb{qg}{h}")
                nc.vector.tensor_copy(zt_sb[0:65, :], zt[0:65, :])
                for qt in range(4):
                    zn = aps_n.tile([128, DV + 1], f16, tag="zn",
                                    name=f"zn{qg}{h}{qt}")
                    nc.tensor.matmul(
                        zn[:],
                        zt_sb[0:65, qt * 128:(qt + 1) * 128],
                        ident[0:65, 0:65],
                        is_transpose=True, start=True, stop=True,
                    )
                    r = rpool.tile([128, 1], f32, tag="r",
                                   name=f"r{qg}{h}{qt}")
                    nc.vector.reciprocal(r[:], zn[:, 64:65])
                    nc.vector.tensor_scalar_mul(
                        z_alls[qt][:, h * DV:(h + 1) * DV],
                        zn[:, 0:DV], r[:],
                    )

            def pv_flush(qg, z_alls):
                for qt in range(4):
                    qrow = qg * QGS + qt * 128
                    nc.sync.dma_start(
                        OUT[qrow:qrow + 128, :], z_alls[qt][:]
                    )

            def z_alloc(qg):
                return [
                    zpool.tile([128, COLS], f32, tag="zall",
                               name=f"za{qg}{qt}")
                    for qt in range(4)
                ]

            for qg in range(1, NQG):
                # interleave PV of the previous q-group with scores of this
                # one per head-pair: frees an eT slot right before each new
                # one is needed, and keeps ScalarE fed with exp throughout
                prev = eT_store.pop(qg - 1)
                za = z_alloc(qg - 1)
                keepT = load_keept(qg)
                cur = []
                for cc in range(2):
                    pv_h(qg - 1, 2 * cc, prev[cc], za)
                    pv_h(qg - 1, 2 * cc + 1, prev[cc], za)
                    cur.append(scores_pair(qg, cc, keepT))
                pv_flush(qg - 1, za)
                eT_store[qg] = cur
            prev = eT_store.pop(NQG - 1)
            za = z_alloc(NQG - 1)
            for cc in range(2):
                pv_h(NQG - 1, 2 * cc, prev[cc], za)
                pv_h(NQG - 1, 2 * cc + 1, prev[cc], za)
            pv_flush(NQG - 1, za)

    nc.compile()
    return nc


def _get_nc():
    if "nc" not in _CACHE:
        _CACHE["nc"] = _build()
    return _CACHE["nc"]


def kernel(x_q, x_k_v, attn_mask, Wq, bq, Wk, bk, Wv, bv):
    x_q = np.asarray(x_q, dtype=np.float32)
    x_k_v = np.asarray(x_k_v, dtype=np.float32)
    attn_mask = np.asarray(attn_mask).astype(bool)
    Wq = np.asarray(Wq, dtype=np.float32)
    Wk = np.asarray(Wk, dtype=np.float32)
    Wv = np.asarray(Wv, dtype=np.float32)
    bq = np.asarray(bq, dtype=np.float32)
    bk = np.asarray(bk, dtype=np.float32)
    bv = np.asarray(bv, dtype=np.float32)

    nc = _get_nc()

    xqt = [np.ascontiguousarray(x_q[b].T).astype(np.float16) for b in range(B)]
    xkvt = [np.ascontiguousarray(x_k_v[b].T).astype(np.float16)
            for b in range(B)]
    keept = [np.ascontiguousarray((~attn_mask[b]).T).astype(np.float16)
             for b in range(B)]

    in_maps = []
    for c in range(NCORE):
        b, g = divmod(c, 4)
        cols = slice(g * COLS, (g + 1) * COLS)
        in_maps.append({
            "xqt": xqt[b],
            "xkvt": xkvt[b],
            "keept": keept[b],
            "wq": np.ascontiguousarray(Wq[:, cols]).astype(np.float16),
            "wk": np.ascontiguousarray(Wk[:, cols]).astype(np.float16),
            "wv": np.ascontiguousarray(Wv[:, cols]).astype(np.float16),
            "bq": np.ascontiguousarray(bq[cols]),
            "bk": np.ascontiguousarray(bk[cols]),
            "bv": np.ascontiguousarray(bv[cols]),
        })

    res = run_bass_kernel_spmd(
        nc, in_maps, core_ids=list(range(NCORE)), trace=TRACE
    )
    if TRACE:
        _CACHE["last_results"] = res

    out = np.empty((B, S, H * DV), np.float32)
    for c in range(NCORE):
        b, g = divmod(c, 4)
        out[b, :, g * COLS:(g + 1) * COLS] = res.results[c]["out"]
    return out
